# revision 39
# baseline (speedup 1.0000x reference)
"""Trainium2 Bass kernel for nn_AttentionBlock (B=2, D=512, N0=N1=2048, H=8).

Sharding: batch (2) x query-position blocks (4) -> 8 cores. Each core
computes the full attention block for one batch and a 512-position query
slice; K/V are computed locally for the whole key sequence, so there are
no collectives and the host gather is a pure concat/transpose.

Per-core layout (channel-major [c, n] everywhere, head channels
permuted to be contiguous on device):
  - K proj:  k[d', m]   = Wk[perm] @ fk        (lhsT = Wk[perm].T)
  - V^T:     vt[m, dv'] = (fk.T @ Wf[perm].T)  with per-head ones column
             appended and *masked rows zeroed* (folds both the softmax
             mask and the denominator into the PV matmul)
  - Q proj:  q[d', n]
  - scores:  S^T[m, n] = k_h^T q_h per head, two heads run concurrently
             in the PE array via 64-row tile_position pairing
  - e = exp(S^T / 8)  (ACT, two m-blocks per call to amortize overhead)
  - PV:      pv'[65, n] = [v_h | 1]^T masked @ e  (row 64 = softmax denom)
  - normalize pv by replicated 1/denom, Wm matmul -> out^T[n, o],
    + (skip + bm), LayerNorm over free axis, store [n, c] blocks.
"""

import os
from contextlib import ExitStack

import numpy as np
import ml_dtypes

import concourse.bass as bass
import concourse.tile as tile
from concourse import bacc, mybir
from concourse.bass_utils import run_bass_kernel_spmd

BF = mybir.dt.bfloat16
F32 = mybir.dt.float32
AF = mybir.ActivationFunctionType

B, D, N0, N1, H = 2, 512, 2048, 2048, 8
HD = 64           # head dim (att and out)
NCORES = 8
P = 128
N0C = N0 // 4     # query positions per core
LN_EPS = 1e-5
SCALE = 1.0 / (1.0 * HD ** 0.5)   # 1/(TEMP * sqrt(head_att))

BF_NP = ml_dtypes.bfloat16


def emit_kernel(ctx: ExitStack, tc, y, ins, n1=N1, n0c=N0C):
    nc = tc.nc
    MB = n1 // P          # m-blocks over keys
    NB = n0c // P         # n-blocks over queries
    G = MB // 2           # exp groups (2 m-blocks per ACT call)
    DB = D // P           # channel blocks
    MCW = min(512, n1)    # proj m-chunk width
    NW = n0c              # score free width (<=512)
    assert MB % 2 == 0 and NW <= 512

    cp = ctx.enter_context(tc.tile_pool(name="consts", bufs=1))
    wp = ctx.enter_context(tc.tile_pool(name="work", bufs=1))
    ep = ctx.enter_context(tc.tile_pool(name="epool", bufs=max(8, 2 * G + 6)))
    npool = ctx.enter_context(tc.tile_pool(name="npool", bufs=2))
    rrpool = ctx.enter_context(tc.tile_pool(name="rrpool", bufs=2))
    xpool = ctx.enter_context(tc.tile_pool(name="xpool", bufs=1))
    sqpool = ctx.enter_context(tc.tile_pool(name="sqpool", bufs=2))
    stat = ctx.enter_context(tc.tile_pool(name="stat", bufs=1))
    opool = ctx.enter_context(tc.tile_pool(name="opool", bufs=1))
    stp = ctx.enter_context(tc.tile_pool(name="stp", bufs=2, space="PSUM"))
    pvp = ctx.enter_context(tc.tile_pool(name="pvp", bufs=2, space="PSUM"))
    mmp = ctx.enter_context(tc.tile_pool(name="mmp", bufs=2, space="PSUM"))

    def load(name, src, shape, dtype, rows=P, eng=None):
        t = cp.tile(shape, dtype, name=name, tag=name)
        (eng or nc.sync).dma_start(t[0:rows, :], src)
        return t

    # DMA order = first-use order. The key stream (fk, 2MB) rides the
    # SWDGE queue so the weight/query stream on HWDGE is uncontended.
    wk_t = cp.tile([P, DB * D], BF, name="wkt", tag="wkt")
    for i in range(DB):
        nc.sync.dma_start(wk_t[:, i * D:(i + 1) * D], ins["wkT"][:, i * D:(i + 1) * D])
    fk_t = []
    for i in range(DB):
        t = cp.tile([P, n1], BF, name=f"fk{i}", tag=f"fk{i}")
        nc.gpsimd.dma_start(t[:, 0:n1 // 2], ins["fk"][i][:, 0:n1 // 2])
        nc.gpsimd.dma_start(t[:, n1 // 2:n1], ins["fk"][i][:, n1 // 2:n1])
        fk_t.append(t)
    wq_t = load("wqt", ins["wqT"], [P, DB * D], BF)
    fqv_t = load("fqvt", ins["fq"], [P, DB * n0c], BF)
    bq_t = load("bqt", ins["bq"], [P, DB], F32)
    bk_t = load("bkt", ins["bk"], [P, DB], F32)
    wf_t = load("wft", ins["wfT"], [P, DB * D], BF)
    mk_t = load("mkt", ins["maskmul"], [P, MB], F32)
    wm_t = load("wmt", ins["wmT"], [P, H * D], BF, rows=HD, eng=nc.gpsimd)
    fqt_t = load("fqtt", ins["fqt"], [P, NB * D], F32, eng=nc.gpsimd)
    lng = load("lng", ins["lng"], [P, D], F32, eng=nc.gpsimd)
    lnb = load("lnb", ins["lnb"], [P, D], F32, eng=nc.gpsimd)

    wk = [wk_t[:, i * D:(i + 1) * D] for i in range(DB)]
    wq = [wq_t[:, i * D:(i + 1) * D] for i in range(DB)]
    wf = [wf_t[:, i * D:(i + 1) * D] for i in range(DB)]
    fqv = [fqv_t[:, i * n0c:(i + 1) * n0c] for i in range(DB)]
    wm = [wm_t[:, h * D:(h + 1) * D] for h in range(H)]
    fqt = [fqt_t[:, i * D:(i + 1) * D] for i in range(NB)]

    ones_bf = cp.tile([P, HD], BF, name="ones", tag="ones")
    nc.vector.memset(ones_bf[:], 1.0)
    epsb = cp.tile([P, 1], F32, name="epsb", tag="epsb")
    nc.vector.memset(epsb[:], LN_EPS)

    k_sb = [wp.tile([P, n1], BF, name=f"ksb{d}", tag=f"ksb{d}") for d in range(DB)]
    q_sb = [wp.tile([P, n0c], BF, name=f"qsb{d}", tag=f"qsb{d}") for d in range(DB)]
    vt_sb = [wp.tile([P, 65 * H], BF, name=f"vt{m}", tag=f"vt{m}") for m in range(MB)]
    pv_sb = [wp.tile([P, n0c], BF, name=f"pvs{h}", tag=f"pvs{h}") for h in range(H)]
    wmacc = [wp.tile([P, D], F32, name=f"wma{nb}", tag=f"wma{nb}")
             for nb in range(NB)]

    e_tiles = {}

    def kproj(db):
        for mc in range(n1 // MCW):
            t = mmp.tile([P, 512], F32, name="mps", tag="mps")
            for ib in range(DB):
                nc.tensor.matmul(
                    t[:, 0:MCW],
                    wk[ib][:, db * P:(db + 1) * P],
                    fk_t[ib][:, mc * MCW:(mc + 1) * MCW],
                    start=(ib == 0), stop=(ib == DB - 1),
                )
            nc.vector.tensor_scalar_add(
                k_sb[db][:, mc * MCW:(mc + 1) * MCW], t[:, 0:MCW],
                bk_t[:, db:db + 1])

    def qproj(db):
        t = mmp.tile([P, 512], F32, name="mps", tag="mps")
        for ib in range(DB):
            nc.tensor.matmul(
                t[:, 0:n0c],
                wq[ib][:, db * P:(db + 1) * P],
                fqv[ib][:],
                start=(ib == 0), stop=(ib == DB - 1),
            )
        nc.vector.tensor_scalar_add(q_sb[db][:], t[:, 0:n0c], bq_t[:, db:db + 1])

    def vtproj(mb):
        t = mmp.tile([P, 512], F32, name="mps", tag="mps")
        for ib in range(DB):
            nc.tensor.matmul(
                t[:],
                fk_t[ib][:, mb * P:(mb + 1) * P],
                wf[ib][:],
                start=(ib == 0), stop=(ib == DB - 1),
            )
        # fk columns are pre-masked on the host, so masked keys already have
        # v == 0 here (bf is folded into fqt host-side since probs sum to 1);
        # only the ones-column needs the mask values.
        r = vt_sb[mb][:].rearrange("p (h c) -> p h c", h=H)
        src = t[:].rearrange("p (h c) -> p h c", h=H)
        nc.vector.tensor_copy(r[:, :, 0:HD], src)
        nc.vector.memset(r[:, :, HD:HD + 1], 1.0)
        nc.vector.tensor_scalar_mul(r[:, :, HD:HD + 1], r[:, :, HD:HD + 1],
                                    mk_t[:, mb:mb + 1])

    def qk_group(p, g):
        sts = []
        for hi in (0, 1):
            lo, hi_p = hi * HD, hi * HD + HD
            st = stp.tile([P, 1024], F32, name="st", tag="st")
            for i in (0, 1):
                mb = 2 * g + i
                nc.tensor.matmul(
                    st[:, i * NW:(i + 1) * NW],
                    k_sb[p][lo:hi_p, mb * P:(mb + 1) * P],
                    q_sb[p][lo:hi_p, :],
                    start=True, stop=True,
                )
            sts.append(st)
        for hi in (0, 1):
            e_t = ep.tile([P, 1024], BF, name="et", tag="et")
            nc.scalar.activation(e_t[:, 0:2 * NW], sts[hi][:, 0:2 * NW],
                                 AF.Exp, scale=SCALE)
            e_tiles[(p, hi, g)] = e_t

    def pv_chunk(p, hi, pvt, mbs):
        h = 2 * p + hi
        for mb in mbs:
            g, i = mb // 2, mb % 2
            e_t = e_tiles[(p, hi, g)]
            nc.tensor.matmul(
                pvt[0:HD + 1, 0:NW],
                vt_sb[mb][:, 65 * h:65 * h + 65],
                e_t[:, i * NW:(i + 1) * NW],
                start=(mb == 0), stop=(mb == MB - 1),
                skip_group_check=True,
            )

    def finish_head(p, hi, pvt):
        h = 2 * p + hi
        nr = npool.tile([P, 512], BF, name="nr", tag="nr")
        with nc.allow_low_precision(reason="softmax denom fits bf16"):
            nc.vector.reciprocal(nr[HD:HD + 1, 0:NW], pvt[HD:HD + 1, 0:NW])
        rrp = mmp.tile([P, 512], F32, name="mps", tag="mps")
        nc.tensor.matmul(rrp[0:HD, 0:NW], ones_bf[HD:HD + 1, 0:HD],
                         nr[HD:HD + 1, 0:NW], start=True, stop=True)
        rrs = rrpool.tile([P, 512], F32, name="rrs", tag="rrs")
        nc.vector.tensor_copy(rrs[0:HD, 0:NW], rrp[0:HD, 0:NW])
        nc.vector.tensor_mul(pv_sb[h][0:HD, 0:n0c], pvt[0:HD, 0:NW],
                             rrs[0:HD, 0:NW])

    def finish_pair(p, pvts):
        for hi in (0, 1):
            finish_head(p, hi, pvts[hi])
        wm_pair(p)

    def wm_pair(p):
        # partial Wm for this head pair, accumulated into SBUF (first pair
        # also folds in the skip connection + bm, pre-summed in fqt)
        for nb in range(NB):
            wmp = mmp.tile([P, 512], F32, name="mps", tag="mps")
            for hi in (0, 1):
                h = 2 * p + hi
                nc.tensor.matmul(
                    wmp[:],
                    pv_sb[h][0:HD, nb * P:(nb + 1) * P],
                    wm[h][0:HD, :],
                    start=(hi == 0), stop=(hi == 1),
                    skip_group_check=True,
                )
            if p == 0:
                nc.vector.tensor_add(wmacc[nb][:], wmp[:], fqt[nb][:])
            else:
                nc.vector.tensor_add(wmacc[nb][:], wmp[:], wmacc[nb][:])

    # ---- emission schedule (PE is in-order; interleave fillers) ----
    # PV for pair p-1 fills pair p's group loop (its deps are a whole pair
    # old, so it never stalls the in-order PE stream).
    kproj(0)
    qproj(0)
    pvts = None
    for p in range(4):
        for g in range(G):
            qk_group(p, g)
            if p == 0:
                vtproj(2 * g)
                vtproj(2 * g + 1)
            else:
                if g == 0:
                    pvts = (pvp.tile([P, 512], F32, name="pvt", tag="pvt"),
                            pvp.tile([P, 512], F32, name="pvt", tag="pvt"))
                pv_chunk(p - 1, 0, pvts[0], [2 * g, 2 * g + 1])
                pv_chunk(p - 1, 1, pvts[1], [2 * g, 2 * g + 1])
        if p >= 1:
            finish_pair(p - 1, pvts)
        if p < 3:
            kproj(p + 1)
            qproj(p + 1)
    pvts = (pvp.tile([P, 512], F32, name="pvt", tag="pvt"),
            pvp.tile([P, 512], F32, name="pvt", tag="pvt"))
    pv_chunk(3, 0, pvts[0], list(range(MB)))
    finish_head(3, 0, pvts[0])
    pv_chunk(3, 1, pvts[1], list(range(MB)))
    finish_head(3, 1, pvts[1])
    wm_pair(3)

    # ---- LayerNorm epilogue (wmacc already holds Wm-out + skip + bm) ----
    # bn_stats/bn_aggr give mean+var in one DVE pass; phase-major emission
    # keeps the in-order DVE/ACT streams dense across n-blocks.
    bnagg_t, std_t, rstd_t, o_t = [], [], [], []
    for nb in range(NB):
        bnst = stat.tile([P, 6], F32, name="bnst", tag=f"bnst{nb}")
        nc.vector.bn_stats(bnst[:], wmacc[nb][:])
        bnagg = stat.tile([P, 2], F32, name="bnagg", tag=f"bnagg{nb}")
        nc.vector.bn_aggr(bnagg[:], bnst[:])
        bnagg_t.append(bnagg)
    for nb in range(NB):
        std = stat.tile([P, 1], F32, name="std", tag=f"std{nb}")
        nc.scalar.activation(std[:], bnagg_t[nb][:, 1:2], AF.Sqrt,
                             bias=epsb[:])
        std_t.append(std)
    for nb in range(NB):
        rstd = stat.tile([P, 1], F32, name="rstd", tag=f"rstd{nb}")
        nc.vector.reciprocal(rstd[:], std_t[nb][:])
        rstd_t.append(rstd)
    o_all = opool.tile([P, NB * D], F32, name="oall", tag="oall")
    for nb in range(NB):
        o = o_all[:, nb * D:(nb + 1) * D]
        nc.vector.tensor_scalar(o, wmacc[nb][:], bnagg_t[nb][:, 0:1],
                                rstd_t[nb][:],
                                op0=mybir.AluOpType.subtract,
                                op1=mybir.AluOpType.mult)
        nc.vector.tensor_mul(o, o, lng[:])
        nc.vector.tensor_add(o, o, lnb[:])
        nc.sync.dma_start(y[:, nb * D:(nb + 1) * D], o)


def build(n1=N1, n0c=N0C):
    MB, NB = n1 // P, n0c // P
    nc = bacc.Bacc("TRN2", target_bir_lowering=False, debug=False,
                   num_devices=NCORES)
    ins = {}

    def din(name, shape, dtype):
        ins[name] = nc.dram_tensor(name, shape, dtype, kind="ExternalInput").ap()

    DBv = D // P
    din("fk", [DBv, P, n1], BF)
    din("fq", [P, DBv * n0c], BF)
    din("fqt", [P, NB * D], F32)
    din("wkT", [P, DBv * D], BF)
    din("wqT", [P, DBv * D], BF)
    din("wfT", [P, DBv * D], BF)
    din("wmT", [HD, H * D], BF)
    din("bq", [P, DBv], F32)
    din("bk", [P, DBv], F32)
    din("maskmul", [P, MB], F32)
    din("lng", [P, D], F32)
    din("lnb", [P, D], F32)
    y = nc.dram_tensor("y", [P, NB * D], F32, kind="ExternalOutput").ap()
    with tile.TileContext(nc) as tc:
        with ExitStack() as ctx:
            emit_kernel(ctx, tc, y, ins, n1=n1, n0c=n0c)
    nc.compile()
    return nc


# device channel d' = h*HD + j  <-  reference channel c = j*H + h
PERM = np.array([j * H + h for h in range(H) for j in range(HD)])


def host_inputs(feats_query, feats_key, key_mask, Wq, bq, Wk, bk, Wf, bf,
                Wm, bm, ln_g, ln_b, n1=N1, n0c=N0C, cores=NCORES):
    MB = n1 // P
    f32 = np.float32
    fq_all = np.asarray(feats_query, f32)
    fk_all = np.asarray(feats_key, f32)
    mask = np.asarray(key_mask)
    Wq, Wk, Wf, Wm = (np.asarray(a, f32) for a in (Wq, Wk, Wf, Wm))
    bq, bk, bf, bm = (np.asarray(a, f32) for a in (bq, bk, bf, bm))
    ln_g, ln_b = np.asarray(ln_g, f32), np.asarray(ln_b, f32)

    def c2(a):  # contiguous f32
        return np.ascontiguousarray(a, dtype=f32)

    def cb(a):  # contiguous bf16
        return np.ascontiguousarray(a).astype(BF_NP)

    DBv = D // P

    def pack(a, rows):  # [nblk(*rows), rows, cols] -> [rows, nblk*cols]
        if a.ndim == 2:
            a = a.reshape(-1, rows, a.shape[1])
        return a.transpose(1, 0, 2).reshape(rows, -1)

    shared = {
        "wkT": cb(pack(np.ascontiguousarray(Wk[PERM].T), P)),
        "wqT": cb(pack(np.ascontiguousarray(Wq[PERM].T), P)),
        "wfT": cb(pack(np.ascontiguousarray(Wf[PERM].T), P)),
        "wmT": cb(pack(np.ascontiguousarray(Wm[:, PERM].T).reshape(H * HD, D),
                       HD)),
        "bq": c2(bq[PERM].reshape(DBv, P).T),
        "bk": c2(bk[PERM].reshape(DBv, P).T),
        "lng": c2(np.broadcast_to(ln_g, (P, D))),
        "lnb": c2(np.broadcast_to(ln_b, (P, D))),
    }
    nslices = cores // fq_all.shape[0]
    in_maps = []
    for c in range(cores):
        b, j = c // nslices, c % nslices
        sl = slice(n0c * j, n0c * (j + 1))
        fq_c = fq_all[b][:, sl]
        mvals = (mask[b, 0] != 0).astype(f32)
        # bf contributes exactly Wm @ bf to the pre-LN output (probs sum
        # to 1), so it folds into the skip/bias tile together with bm.
        skip_bias = bm + Wm @ bf
        m = {
            # pre-masked keys: masked positions get k == v == 0 on device
            "fk": cb(fk_all[b] * mvals[None, :]).reshape(DBv, P, n1),
            "fq": cb(pack(fq_c.reshape(DBv, P, n0c), P)),
            "fqt": c2(pack((fq_c.T + skip_bias[None, :]).reshape(
                n0c // P, P, D), P)),
            "maskmul": c2(mvals.reshape(MB, P).T),
        }
        m.update(shared)
        in_maps.append(m)
    return in_maps


_NC_CACHE = {}


def kernel(**inputs):
    key = "full"
    if key not in _NC_CACHE:
        _NC_CACHE[key] = build()
    nc = _NC_CACHE[key]
    in_maps = host_inputs(**inputs)
    res = run_bass_kernel_spmd(nc, in_maps, core_ids=list(range(NCORES)))
    out = np.empty((B, D, N0), dtype=np.float32)
    nslices = NCORES // B
    for c in range(NCORES):
        b, j = c // nslices, c % nslices
        o = res.results[c]["y"].reshape(P, N0C // P, D).transpose(
            1, 0, 2).reshape(N0C, D)
        out[b][:, N0C * j:N0C * (j + 1)] = o.T
    return out


if __name__ == "__main__":
    import json
    rng = np.random.default_rng(0)
    ins = {
        "feats_query": rng.normal(size=(B, D, N0)).astype(np.float32),
        "feats_key": rng.normal(size=(B, D, N1)).astype(np.float32),
        "key_mask": rng.integers(0, 2, size=(B, 1, N1)).astype(np.int32),
        "Wq": (rng.normal(size=(D, D)) * 0.02).astype(np.float32),
        "bq": np.zeros(D, np.float32),
        "Wk": (rng.normal(size=(D, D)) * 0.02).astype(np.float32),
        "bk": np.zeros(D, np.float32),
        "Wf": (rng.normal(size=(D, D)) * 0.02).astype(np.float32),
        "bf": np.zeros(D, np.float32),
        "Wm": (rng.normal(size=(D, D)) * 0.02).astype(np.float32),
        "bm": np.zeros(D, np.float32),
        "ln_g": np.ones(D, np.float32),
        "ln_b": np.zeros(D, np.float32),
    }
    out = kernel(**ins)
    print("out", out.shape, out.dtype, float(np.abs(out).mean()))


# revision 44
# speedup vs baseline: 1.0170x; 1.0170x over previous
"""Trainium2 Bass kernel for nn_AttentionBlock (B=2, D=512, N0=N1=2048, H=8).

Sharding: batch (2) x query-position blocks (4) -> 8 cores. Each core
computes the full attention block for one batch and a 512-position query
slice; K/V are computed locally for the whole key sequence, so there are
no collectives and the host gather is a pure concat/transpose.

Per-core layout (channel-major [c, n] everywhere, head channels
permuted to be contiguous on device):
  - K proj:  k[d', m]   = Wk[perm] @ fk        (lhsT = Wk[perm].T)
  - V^T:     vt[m, dv'] = (fk.T @ Wf[perm].T)  with per-head ones column
             appended and *masked rows zeroed* (folds both the softmax
             mask and the denominator into the PV matmul)
  - Q proj:  q[d', n]
  - scores:  S^T[m, n] = k_h^T q_h per head, two heads run concurrently
             in the PE array via 64-row tile_position pairing
  - e = exp(S^T / 8)  (ACT, two m-blocks per call to amortize overhead)
  - PV:      pv'[65, n] = [v_h | 1]^T masked @ e  (row 64 = softmax denom)
  - normalize pv by replicated 1/denom, Wm matmul -> out^T[n, o],
    + (skip + bm), LayerNorm over free axis, store [n, c] blocks.
"""

import os
from contextlib import ExitStack

import numpy as np
import ml_dtypes

import concourse.bass as bass
import concourse.tile as tile
from concourse import bacc, mybir
from concourse.bass_utils import run_bass_kernel_spmd

BF = mybir.dt.bfloat16
F32 = mybir.dt.float32
AF = mybir.ActivationFunctionType

B, D, N0, N1, H = 2, 512, 2048, 2048, 8
HD = 64           # head dim (att and out)
NCORES = 8
P = 128
N0C = N0 // 4     # query positions per core
LN_EPS = 1e-5
SCALE = 1.0 / (1.0 * HD ** 0.5)   # 1/(TEMP * sqrt(head_att))

BF_NP = ml_dtypes.bfloat16


def emit_kernel(ctx: ExitStack, tc, y, ins, n1=N1, n0c=N0C):
    nc = tc.nc
    MB = n1 // P          # m-blocks over keys
    NB = n0c // P         # n-blocks over queries
    G = MB // 2           # exp groups (2 m-blocks per ACT call)
    DB = D // P           # channel blocks
    MCW = min(512, n1)    # proj m-chunk width
    NW = n0c              # score free width (<=512)
    assert MB % 2 == 0 and NW <= 512

    cp = ctx.enter_context(tc.tile_pool(name="consts", bufs=1))
    wp = ctx.enter_context(tc.tile_pool(name="work", bufs=1))
    ep = ctx.enter_context(tc.tile_pool(name="epool", bufs=max(8, 2 * G + 6)))
    npool = ctx.enter_context(tc.tile_pool(name="npool", bufs=2))
    rrpool = ctx.enter_context(tc.tile_pool(name="rrpool", bufs=2))
    xpool = ctx.enter_context(tc.tile_pool(name="xpool", bufs=1))
    sqpool = ctx.enter_context(tc.tile_pool(name="sqpool", bufs=2))
    stat = ctx.enter_context(tc.tile_pool(name="stat", bufs=1))
    opool = ctx.enter_context(tc.tile_pool(name="opool", bufs=1))
    stp = ctx.enter_context(tc.tile_pool(name="stp", bufs=2, space="PSUM"))
    pvp = ctx.enter_context(tc.tile_pool(name="pvp", bufs=2, space="PSUM"))
    mmp = ctx.enter_context(tc.tile_pool(name="mmp", bufs=2, space="PSUM"))

    def load(name, src, shape, dtype, rows=P, eng=None):
        t = cp.tile(shape, dtype, name=name, tag=name)
        (eng or nc.sync).dma_start(t[0:rows, :], src)
        return t

    # DMA order = first-use order. The key stream (fk, 2MB) rides the
    # SWDGE queue so the weight/query stream on HWDGE is uncontended.
    wk_t = cp.tile([P, DB * D], BF, name="wkt", tag="wkt")
    for i in range(DB):
        nc.sync.dma_start(wk_t[:, i * D:(i + 1) * D], ins["wkT"][:, i * D:(i + 1) * D])
    fk_t = [cp.tile([P, n1], BF, name=f"fk{i}", tag=f"fk{i}")
            for i in range(DB)]
    for i in range(DB):
        nc.gpsimd.dma_start(fk_t[i][:, 0:n1 // 2], ins["fk"][i][:, 0:n1 // 2])
    for i in range(DB):
        nc.gpsimd.dma_start(fk_t[i][:, n1 // 2:n1], ins["fk"][i][:, n1 // 2:n1])
    wq_t = load("wqt", ins["wqT"], [P, DB * D], BF)
    fqv_t = load("fqvt", ins["fq"], [P, DB * n0c], BF)
    bq_t = load("bqt", ins["bq"], [P, DB], F32)
    bk_t = load("bkt", ins["bk"], [P, DB], F32)
    wf_t = load("wft", ins["wfT"], [P, DB * D], BF)
    mk_t = load("mkt", ins["maskmul"], [P, MB], F32)
    wm_t = load("wmt", ins["wmT"], [P, H * D], BF, rows=HD, eng=nc.gpsimd)
    fqt_t = load("fqtt", ins["fqt"], [P, NB * D], F32, eng=nc.gpsimd)
    lng = load("lng", ins["lng"], [P, D], F32, eng=nc.gpsimd)
    lnb = load("lnb", ins["lnb"], [P, D], F32, eng=nc.gpsimd)

    wk = [wk_t[:, i * D:(i + 1) * D] for i in range(DB)]
    wq = [wq_t[:, i * D:(i + 1) * D] for i in range(DB)]
    wf = [wf_t[:, i * D:(i + 1) * D] for i in range(DB)]
    fqv = [fqv_t[:, i * n0c:(i + 1) * n0c] for i in range(DB)]
    wm = [wm_t[:, h * D:(h + 1) * D] for h in range(H)]
    fqt = [fqt_t[:, i * D:(i + 1) * D] for i in range(NB)]

    ones_bf = cp.tile([P, HD], BF, name="ones", tag="ones")
    nc.vector.memset(ones_bf[:], 1.0)
    epsb = cp.tile([P, 1], F32, name="epsb", tag="epsb")
    nc.vector.memset(epsb[:], LN_EPS)

    k_sb = [wp.tile([P, n1], BF, name=f"ksb{d}", tag=f"ksb{d}") for d in range(DB)]
    q_sb = [wp.tile([P, n0c], BF, name=f"qsb{d}", tag=f"qsb{d}") for d in range(DB)]
    vt_sb = [wp.tile([P, 65 * H], BF, name=f"vt{m}", tag=f"vt{m}") for m in range(MB)]
    pv_sb = [wp.tile([P, n0c], BF, name=f"pvs{h}", tag=f"pvs{h}") for h in range(H)]
    wmacc = [wp.tile([P, D], F32, name=f"wma{nb}", tag=f"wma{nb}")
             for nb in range(NB)]

    e_tiles = {}

    def kproj(db):
        for mc in range(n1 // MCW):
            t = mmp.tile([P, 512], F32, name="mps", tag="mps")
            for ib in range(DB):
                nc.tensor.matmul(
                    t[:, 0:MCW],
                    wk[ib][:, db * P:(db + 1) * P],
                    fk_t[ib][:, mc * MCW:(mc + 1) * MCW],
                    start=(ib == 0), stop=(ib == DB - 1),
                )
            nc.vector.tensor_scalar_add(
                k_sb[db][:, mc * MCW:(mc + 1) * MCW], t[:, 0:MCW],
                bk_t[:, db:db + 1])

    def qproj(db):
        t = mmp.tile([P, 512], F32, name="mps", tag="mps")
        for ib in range(DB):
            nc.tensor.matmul(
                t[:, 0:n0c],
                wq[ib][:, db * P:(db + 1) * P],
                fqv[ib][:],
                start=(ib == 0), stop=(ib == DB - 1),
            )
        nc.vector.tensor_scalar_add(q_sb[db][:], t[:, 0:n0c], bq_t[:, db:db + 1])

    def vtproj(mb):
        t = mmp.tile([P, 512], F32, name="mps", tag="mps")
        for ib in range(DB):
            nc.tensor.matmul(
                t[:],
                fk_t[ib][:, mb * P:(mb + 1) * P],
                wf[ib][:],
                start=(ib == 0), stop=(ib == DB - 1),
            )
        # fk columns are pre-masked on the host, so masked keys already have
        # v == 0 here (bf is folded into fqt host-side since probs sum to 1);
        # only the ones-column needs the mask values.
        r = vt_sb[mb][:].rearrange("p (h c) -> p h c", h=H)
        src = t[:].rearrange("p (h c) -> p h c", h=H)
        nc.vector.tensor_copy(r[:, :, 0:HD], src)
        nc.vector.memset(r[:, :, HD:HD + 1], 1.0)
        nc.vector.tensor_scalar_mul(r[:, :, HD:HD + 1], r[:, :, HD:HD + 1],
                                    mk_t[:, mb:mb + 1])

    def qk_group(p, g):
        # even head on PE rows 0-63, odd head on rows 64-127: keep the two
        # K=64 matmuls adjacent so the row-tiles run concurrently.
        sts = [stp.tile([P, 1024], F32, name="st", tag="st"),
               stp.tile([P, 1024], F32, name="st", tag="st")]
        for i in (0, 1):
            mb = 2 * g + i
            for hi in (0, 1):
                lo, hi_p = hi * HD, hi * HD + HD
                nc.tensor.matmul(
                    sts[hi][:, i * NW:(i + 1) * NW],
                    k_sb[p][lo:hi_p, mb * P:(mb + 1) * P],
                    q_sb[p][lo:hi_p, :],
                    start=True, stop=True,
                )
        for hi in (0, 1):
            e_t = ep.tile([P, 1024], BF, name="et", tag="et")
            nc.scalar.activation(e_t[:, 0:2 * NW], sts[hi][:, 0:2 * NW],
                                 AF.Exp, scale=SCALE)
            e_tiles[(p, hi, g)] = e_t

    def pv_chunk(p, hi, pvt, mbs):
        h = 2 * p + hi
        for mb in mbs:
            g, i = mb // 2, mb % 2
            e_t = e_tiles[(p, hi, g)]
            nc.tensor.matmul(
                pvt[0:HD + 1, 0:NW],
                vt_sb[mb][:, 65 * h:65 * h + 65],
                e_t[:, i * NW:(i + 1) * NW],
                start=(mb == 0), stop=(mb == MB - 1),
                skip_group_check=True,
            )

    def finish_head(p, hi, pvt):
        h = 2 * p + hi
        nr = npool.tile([P, 512], BF, name="nr", tag="nr")
        with nc.allow_low_precision(reason="softmax denom fits bf16"):
            nc.vector.reciprocal(nr[HD:HD + 1, 0:NW], pvt[HD:HD + 1, 0:NW])
        rrp = mmp.tile([P, 512], F32, name="mps", tag="mps")
        nc.tensor.matmul(rrp[0:HD, 0:NW], ones_bf[HD:HD + 1, 0:HD],
                         nr[HD:HD + 1, 0:NW], start=True, stop=True)
        rrs = rrpool.tile([P, 512], F32, name="rrs", tag="rrs")
        nc.vector.tensor_copy(rrs[0:HD, 0:NW], rrp[0:HD, 0:NW])
        nc.vector.tensor_mul(pv_sb[h][0:HD, 0:n0c], pvt[0:HD, 0:NW],
                             rrs[0:HD, 0:NW])

    def finish_pair(p, pvts):
        for hi in (0, 1):
            finish_head(p, hi, pvts[hi])
        wm_pair(p)

    def wm_pair(p):
        # partial Wm for this head pair, accumulated into SBUF (first pair
        # also folds in the skip connection + bm, pre-summed in fqt)
        for nb in range(NB):
            wmp = mmp.tile([P, 512], F32, name="mps", tag="mps")
            for hi in (0, 1):
                h = 2 * p + hi
                nc.tensor.matmul(
                    wmp[:],
                    pv_sb[h][0:HD, nb * P:(nb + 1) * P],
                    wm[h][0:HD, :],
                    start=(hi == 0), stop=(hi == 1),
                    skip_group_check=True,
                )
            if p == 0:
                nc.vector.tensor_add(wmacc[nb][:], wmp[:], fqt[nb][:])
            else:
                nc.vector.tensor_add(wmacc[nb][:], wmp[:], wmacc[nb][:])

    # ---- emission schedule (PE is in-order; interleave fillers) ----
    # Dummy 1-element matmuls on the DMA-free ones tile cover the first
    # input DMA's latency and warm the PE HAM clock gate (~3.4us window).
    warm = mmp.tile([P, 512], F32, name="mps", tag="mps")
    for _ in range(18):
        nc.tensor.matmul(warm[0:1, 0:1], ones_bf[0:1, 0:1], ones_bf[0:1, 0:1],
                         start=True, stop=True)
    # PV for pair p-1 fills pair p's group loop (its deps are a whole pair
    # old, so it never stalls the in-order PE stream).
    kproj(0)
    qproj(0)
    pvts = None
    for p in range(4):
        for g in range(G):
            qk_group(p, g)
            if p == 0:
                vtproj(2 * g)
                vtproj(2 * g + 1)
            else:
                if g == 0:
                    pvts = (pvp.tile([P, 512], F32, name="pvt", tag="pvt"),
                            pvp.tile([P, 512], F32, name="pvt", tag="pvt"))
                pv_chunk(p - 1, 0, pvts[0], [2 * g, 2 * g + 1])
                pv_chunk(p - 1, 1, pvts[1], [2 * g, 2 * g + 1])
        if p >= 1:
            finish_pair(p - 1, pvts)
        if p < 3:
            kproj(p + 1)
            qproj(p + 1)
    pvts = (pvp.tile([P, 512], F32, name="pvt", tag="pvt"),
            pvp.tile([P, 512], F32, name="pvt", tag="pvt"))
    pv_chunk(3, 0, pvts[0], list(range(MB)))
    finish_head(3, 0, pvts[0])
    pv_chunk(3, 1, pvts[1], list(range(MB)))
    finish_head(3, 1, pvts[1])
    wm_pair(3)

    # ---- LayerNorm epilogue (wmacc already holds Wm-out + skip + bm) ----
    # bn_stats/bn_aggr give mean+var in one DVE pass; phase-major emission
    # keeps the in-order DVE/ACT streams dense across n-blocks.
    bnagg_t, std_t, rstd_t, o_t = [], [], [], []
    for nb in range(NB):
        bnst = stat.tile([P, 6], F32, name="bnst", tag=f"bnst{nb}")
        nc.vector.bn_stats(bnst[:], wmacc[nb][:])
        bnagg = stat.tile([P, 2], F32, name="bnagg", tag=f"bnagg{nb}")
        nc.vector.bn_aggr(bnagg[:], bnst[:])
        bnagg_t.append(bnagg)
    for nb in range(NB):
        std = stat.tile([P, 1], F32, name="std", tag=f"std{nb}")
        nc.scalar.activation(std[:], bnagg_t[nb][:, 1:2], AF.Sqrt,
                             bias=epsb[:])
        std_t.append(std)
    for nb in range(NB):
        rstd = stat.tile([P, 1], F32, name="rstd", tag=f"rstd{nb}")
        nc.vector.reciprocal(rstd[:], std_t[nb][:])
        rstd_t.append(rstd)
    o_all = opool.tile([P, NB * D], F32, name="oall", tag="oall")
    for nb in range(NB):
        o = o_all[:, nb * D:(nb + 1) * D]
        nc.vector.tensor_scalar(o, wmacc[nb][:], bnagg_t[nb][:, 0:1],
                                rstd_t[nb][:],
                                op0=mybir.AluOpType.subtract,
                                op1=mybir.AluOpType.mult)
        nc.vector.tensor_mul(o, o, lng[:])
        nc.vector.tensor_add(o, o, lnb[:])
        nc.sync.dma_start(y[:, nb * D:(nb + 1) * D], o)


def build(n1=N1, n0c=N0C):
    MB, NB = n1 // P, n0c // P
    nc = bacc.Bacc("TRN2", target_bir_lowering=False, debug=False,
                   num_devices=NCORES)
    ins = {}

    def din(name, shape, dtype):
        ins[name] = nc.dram_tensor(name, shape, dtype, kind="ExternalInput").ap()

    DBv = D // P
    din("fk", [DBv, P, n1], BF)
    din("fq", [P, DBv * n0c], BF)
    din("fqt", [P, NB * D], F32)
    din("wkT", [P, DBv * D], BF)
    din("wqT", [P, DBv * D], BF)
    din("wfT", [P, DBv * D], BF)
    din("wmT", [HD, H * D], BF)
    din("bq", [P, DBv], F32)
    din("bk", [P, DBv], F32)
    din("maskmul", [P, MB], F32)
    din("lng", [P, D], F32)
    din("lnb", [P, D], F32)
    y = nc.dram_tensor("y", [P, NB * D], F32, kind="ExternalOutput").ap()
    with tile.TileContext(nc) as tc:
        with ExitStack() as ctx:
            emit_kernel(ctx, tc, y, ins, n1=n1, n0c=n0c)
    nc.compile()
    return nc


# device channel d' = h*HD + j  <-  reference channel c = j*H + h
PERM = np.array([j * H + h for h in range(H) for j in range(HD)])


def host_inputs(feats_query, feats_key, key_mask, Wq, bq, Wk, bk, Wf, bf,
                Wm, bm, ln_g, ln_b, n1=N1, n0c=N0C, cores=NCORES):
    MB = n1 // P
    f32 = np.float32
    fq_all = np.asarray(feats_query, f32)
    fk_all = np.asarray(feats_key, f32)
    mask = np.asarray(key_mask)
    Wq, Wk, Wf, Wm = (np.asarray(a, f32) for a in (Wq, Wk, Wf, Wm))
    bq, bk, bf, bm = (np.asarray(a, f32) for a in (bq, bk, bf, bm))
    ln_g, ln_b = np.asarray(ln_g, f32), np.asarray(ln_b, f32)

    def c2(a):  # contiguous f32
        return np.ascontiguousarray(a, dtype=f32)

    def cb(a):  # contiguous bf16
        return np.ascontiguousarray(a).astype(BF_NP)

    DBv = D // P

    def pack(a, rows):  # [nblk(*rows), rows, cols] -> [rows, nblk*cols]
        if a.ndim == 2:
            a = a.reshape(-1, rows, a.shape[1])
        return a.transpose(1, 0, 2).reshape(rows, -1)

    shared = {
        "wkT": cb(pack(np.ascontiguousarray(Wk[PERM].T), P)),
        "wqT": cb(pack(np.ascontiguousarray(Wq[PERM].T), P)),
        "wfT": cb(pack(np.ascontiguousarray(Wf[PERM].T), P)),
        "wmT": cb(pack(np.ascontiguousarray(Wm[:, PERM].T).reshape(H * HD, D),
                       HD)),
        "bq": c2(bq[PERM].reshape(DBv, P).T),
        "bk": c2(bk[PERM].reshape(DBv, P).T),
        "lng": c2(np.broadcast_to(ln_g, (P, D))),
        "lnb": c2(np.broadcast_to(ln_b, (P, D))),
    }
    nslices = cores // fq_all.shape[0]
    in_maps = []
    for c in range(cores):
        b, j = c // nslices, c % nslices
        sl = slice(n0c * j, n0c * (j + 1))
        fq_c = fq_all[b][:, sl]
        mvals = (mask[b, 0] != 0).astype(f32)
        # bf contributes exactly Wm @ bf to the pre-LN output (probs sum
        # to 1), so it folds into the skip/bias tile together with bm.
        skip_bias = bm + Wm @ bf
        m = {
            # pre-masked keys: masked positions get k == v == 0 on device
            "fk": cb(fk_all[b] * mvals[None, :]).reshape(DBv, P, n1),
            "fq": cb(pack(fq_c.reshape(DBv, P, n0c), P)),
            "fqt": c2(pack((fq_c.T + skip_bias[None, :]).reshape(
                n0c // P, P, D), P)),
            "maskmul": c2(mvals.reshape(MB, P).T),
        }
        m.update(shared)
        in_maps.append(m)
    return in_maps


_NC_CACHE = {}


def kernel(**inputs):
    key = "full"
    if key not in _NC_CACHE:
        _NC_CACHE[key] = build()
    nc = _NC_CACHE[key]
    in_maps = host_inputs(**inputs)
    res = run_bass_kernel_spmd(nc, in_maps, core_ids=list(range(NCORES)))
    out = np.empty((B, D, N0), dtype=np.float32)
    nslices = NCORES // B
    for c in range(NCORES):
        b, j = c // nslices, c % nslices
        o = res.results[c]["y"].reshape(P, N0C // P, D).transpose(
            1, 0, 2).reshape(N0C, D)
        out[b][:, N0C * j:N0C * (j + 1)] = o.T
    return out


if __name__ == "__main__":
    import json
    rng = np.random.default_rng(0)
    ins = {
        "feats_query": rng.normal(size=(B, D, N0)).astype(np.float32),
        "feats_key": rng.normal(size=(B, D, N1)).astype(np.float32),
        "key_mask": rng.integers(0, 2, size=(B, 1, N1)).astype(np.int32),
        "Wq": (rng.normal(size=(D, D)) * 0.02).astype(np.float32),
        "bq": np.zeros(D, np.float32),
        "Wk": (rng.normal(size=(D, D)) * 0.02).astype(np.float32),
        "bk": np.zeros(D, np.float32),
        "Wf": (rng.normal(size=(D, D)) * 0.02).astype(np.float32),
        "bf": np.zeros(D, np.float32),
        "Wm": (rng.normal(size=(D, D)) * 0.02).astype(np.float32),
        "bm": np.zeros(D, np.float32),
        "ln_g": np.ones(D, np.float32),
        "ln_b": np.zeros(D, np.float32),
    }
    out = kernel(**ins)
    print("out", out.shape, out.dtype, float(np.abs(out).mean()))
